# revision 13
# baseline (speedup 1.0000x reference)
"""Trainium2 Bass kernel for nn_GroupEncoder — v10 (bf16 HBM, single-queue stream).

Computes, for full inputs
    x:  (32, 128, 128, 128) f32
    r:  (32, 128, 128, 32)  f32
    w1: (128, 32, 8, 16)    f32
    w2: (32, 16, 8, 16)     f32
the reference:
    y = einsum('nijx,nijr->nrx', x, r)
    u = relu(einsum('nrx,xrvh->nrvh', y, w1) / (128*128))
    out = einsum('ruvh,nrvh->nruv', w2, u)        # (32, 32, 16, 8)

Sharding: data-parallel over n across 8 NeuronCores (4 samples/core),
w1/w2 replicated.

Design per core:
  - x/r pre-cast to bf16 host-side; device stream is 21 MB/core
    (memory-bound; correctness gate 2e-2, bf16 gives ~4.5e-3).
  - SDMA busy-efficiency tracks per-partition descriptor length, and
    the SWDGE Q7 descriptor generator needs ~5us per transfer: so x
    moves as whole-sample 4 MiB transfers (32 KB rows, 6 transfers
    total) on the gpsimd/SWDGE queue, which profiling showed stays
    balanced across all 16 SDMA engines.  The last sample tapers
    2+1+1 MiB so only ~32 matmuls sit behind the final transfer.
    r (1 MiB per sample) rides the same queue in strict consumption
    order (r_n, x_n, ...): profiling showed the single-queue SWDGE
    stream is the only configuration with zero transfer-boundary gaps
    and no per-engine straggler; no DMA queue carries compute.
  - Stage 1: per sample 128 bf16 matmuls (K=i on partitions,
    x stationary, r moving N=32) accumulating y^T in PSUM (f32);
    evacuation on DVE.
  - The head (stage 2 + relu + stage 3 + output) runs at the end in
    two rr-halves with each stage in its own PSUM bank, so DVE work
    on half k genuinely overlaps PE work on half k+1 (single-bank
    staging would be serialized by the bank-overlap tracker).
"""

import numpy as np

# Problem constants (hardcoded; kernel.py must be self-contained).
N, I, J = 32, 128, 128
XD, RD, UD, VD, HD = 128, 32, 16, 8, 16
NCORES = 8
NLOC = N // NCORES  # 4 samples per core
NORM = float(I * J)

# x chunking: j-extents per DMA, per sample; final sample tapers.
CHUNKS = [[128], [128], [128], [64, 32, 32]]

_cache = {}


def _build_nc():
    import concourse.mybir as mybir
    import concourse.tile as tile
    from concourse import bacc

    f32 = mybir.dt.float32
    bf16 = mybir.dt.bfloat16

    nc = bacc.Bacc(
        "TRN2", target_bir_lowering=False, debug=False, num_devices=NCORES
    )
    x_d = nc.dram_tensor("x", [NLOC, I, J * XD], bf16, kind="ExternalInput").ap()
    r_d = nc.dram_tensor("r", [NLOC, I, J * RD], bf16, kind="ExternalInput").ap()
    w1_d = nc.dram_tensor("w1", [XD, RD * VD * HD], bf16, kind="ExternalInput").ap()
    w2_d = nc.dram_tensor(
        "w2bd", [VD * HD, RD * UD * VD], bf16, kind="ExternalInput"
    ).ap()
    out_d = nc.dram_tensor(
        "out", [UD * VD, RD * NLOC], f32, kind="ExternalOutput"
    ).ap()

    with tile.TileContext(nc) as tc:
        with (
            tc.tile_pool(name="xp", bufs=3) as xp,
            tc.tile_pool(name="rp", bufs=2) as rp,
            tc.tile_pool(name="wp", bufs=1) as wp,
            tc.tile_pool(name="pys", bufs=2, space="PSUM") as pys,
            tc.tile_pool(name="pu1", bufs=2, space="PSUM") as pu1,
            tc.tile_pool(name="pu2", bufs=2, space="PSUM") as pu2,
        ):
            w1_sb = wp.tile([XD, RD * VD * HD], bf16)
            nc.scalar.dma_start(w1_sb[:, :], w1_d[:, :])
            w2_sb = wp.tile([VD * HD, RD * UD * VD], bf16)
            nc.scalar.dma_start(w2_sb[:, :], w2_d[:, :])
            # y^T staging: [x, (rr n)] with column rr*NLOC + n, bf16
            yT_sb = wp.tile([XD, RD * NLOC], bf16)

            for n in range(NLOC):
                ypsum = pys.tile([XD, RD], f32)
                rt = rp.tile([I, J * RD], bf16)
                nc.gpsimd.dma_start(rt[:, :], r_d[n, :, :])
                j0 = 0
                for jc in CHUNKS[n]:
                    xt = xp.tile([I, jc * XD], bf16, tag="xt")
                    nc.gpsimd.dma_start(
                        xt[:, :], x_d[n, :, j0 * XD : (j0 + jc) * XD]
                    )
                    for j in range(jc):
                        jj = j0 + j
                        nc.tensor.matmul(
                            ypsum[:, :],
                            xt[:, j * XD : (j + 1) * XD],
                            rt[:, jj * RD : (jj + 1) * RD],
                            start=(jj == 0),
                            stop=(jj == J - 1),
                        )
                    j0 += jc
                # Evacuate y^T for this sample on DVE (strided dst rr*NLOC+n)
                nc.vector.tensor_copy(
                    yT_sb[:, n : RD * NLOC : NLOC], ypsum[:, :]
                )

            # Head in two rr-halves, each stage in its own PSUM bank so
            # halves pipeline across PE and DVE.
            u1_sb = wp.tile([VD * HD, RD * NLOC], bf16)
            out_sb = wp.tile([UD * VD, RD * NLOC], f32)
            H = RD // 2
            for half in range(2):
                lo, hi = half * H, (half + 1) * H
                # Stage 2: u1[vh, (rr n)] = sum_x w1[x,(rr vh)] yT[x,(rr n)]
                u1ps = pu1.tile([VD * HD, H * NLOC], f32, tag="u1")
                for rr in range(lo, hi):
                    nc.tensor.matmul(
                        u1ps[:, (rr - lo) * NLOC : (rr - lo + 1) * NLOC],
                        w1_sb[:, rr * VD * HD : (rr + 1) * VD * HD],
                        yT_sb[:, rr * NLOC : (rr + 1) * NLOC],
                        start=True,
                        stop=True,
                    )
                # relu on DVE, cast to bf16 for stage 3
                nc.vector.tensor_scalar_max(
                    u1_sb[:, lo * NLOC : hi * NLOC], u1ps[:, :], 0.0
                )
                # Stage 3: u2[uv, (rr n)] = sum_vh w2bd[vh,(rr uv)] u1[...]
                u2ps = pu2.tile([UD * VD, H * NLOC], f32, tag="u2")
                for rr in range(lo, hi):
                    nc.tensor.matmul(
                        u2ps[:, (rr - lo) * NLOC : (rr - lo + 1) * NLOC],
                        w2_sb[:, rr * UD * VD : (rr + 1) * UD * VD],
                        u1_sb[:, rr * NLOC : (rr + 1) * NLOC],
                        start=True,
                        stop=True,
                    )
                nc.vector.tensor_copy(
                    out_sb[:, lo * NLOC : hi * NLOC], u2ps[:, :]
                )
                nc.sync.dma_start(
                    out_d[:, lo * NLOC : hi * NLOC],
                    out_sb[:, lo * NLOC : hi * NLOC],
                )

    nc.compile()
    return nc


def _prep_in_maps(x, r, w1, w2):
    import ml_dtypes

    bf = ml_dtypes.bfloat16
    x = np.asarray(x, dtype=np.float32)
    r = np.asarray(r, dtype=np.float32)
    w1 = np.asarray(w1, dtype=np.float32)
    w2 = np.asarray(w2, dtype=np.float32)

    # Fold the 1/(i*j) normalization into w1.
    w1p = np.ascontiguousarray((w1 / NORM).reshape(XD, RD * VD * HD)).astype(bf)
    # Block-diagonal expansion of w2 over v:
    # w2bd[(v h), r, (u v')] = w2[r, u, v, h] if v == v' else 0
    w2bd = np.zeros((RD, VD, HD, UD, VD), np.float32)
    for v in range(VD):
        w2bd[:, v, :, :, v] = np.transpose(w2[:, :, v, :], (0, 2, 1))
    w2bd = np.ascontiguousarray(
        w2bd.reshape(RD, VD * HD, UD * VD)
        .transpose(1, 0, 2)
        .reshape(VD * HD, RD * UD * VD)
    ).astype(bf)

    xb = x.astype(bf)
    rb = r.astype(bf)
    in_maps = []
    for c in range(NCORES):
        in_maps.append(
            {
                "x": np.ascontiguousarray(
                    xb[c * NLOC : (c + 1) * NLOC].reshape(NLOC, I, J * XD)
                ),
                "r": np.ascontiguousarray(
                    rb[c * NLOC : (c + 1) * NLOC].reshape(NLOC, I, J * RD)
                ),
                "w1": w1p,
                "w2bd": w2bd,
            }
        )
    return in_maps


def _assemble(results):
    outs = []
    for c in range(NCORES):
        o = np.asarray(results[c]["out"], dtype=np.float32)  # [uv, (rr n)]
        outs.append(o.reshape(UD, VD, RD, NLOC).transpose(3, 2, 0, 1))
    return np.ascontiguousarray(np.concatenate(outs, axis=0))


def run(x, r, w1, w2, **spmd_kwargs):
    """Build (cached), run on 8 cores, return (output, BassKernelResults)."""
    from concourse.bass_utils import run_bass_kernel_spmd

    if "nc" not in _cache:
        _cache["nc"] = _build_nc()
    nc = _cache["nc"]
    in_maps = _prep_in_maps(x, r, w1, w2)
    res = run_bass_kernel_spmd(
        nc, in_maps, core_ids=list(range(NCORES)), **spmd_kwargs
    )
    return _assemble(res.results), res


def kernel(x, r, w1, w2):
    out, _ = run(x, r, w1, w2)
    return out
